# revision 3
# baseline (speedup 1.0000x reference)
"""Trainium2 Bass kernel for nn_CIFAR_SSM_Classifier.

Data-parallel over 8 NeuronCores: each core processes 64 of the 512 images.

Per-core pipeline (all resident in SBUF, fp32r matmuls on the PE):
  conv1 (3->64, 3x3)  : 9 taps stacked on K (27 rows), 1 matmul per 512-col bank
  conv2 (64->128,3x3) : 3 col-adjacent tap pairs stacked to K=128 (via a
                        shifted copy of fm1 in partitions 64-127) + 3 singles
  conv3 (128->128,3x3): 9 matmuls of K=128 per bank
  BN+ReLU fused into the PSUM->SBUF drain on the Scalar engine.
  Feature maps use a zero-padded 34x34 frame layout per image so all 9 conv
  taps are plain column offsets of one SBUF tile.
  width-mean -> SSM: the L=32 diagonal scan is unrolled algebraically:
    sum_t x_t = sum_tau w_tau (.) (B u_tau),  w_tau = sum_{k<=L-1-tau} A^k
  so the whole SSM+head collapses into a few small matmuls + one weighted
  reduction. BN folding and the geometric weights are precomputed host-side
  (numpy) from the parameter tensors.
"""
import numpy as np

import concourse.bass as bass
import concourse.tile as tile
from concourse import bacc, mybir
from concourse.bass_utils import run_bass_kernel_spmd
from concourse.masks import make_identity

F32 = mybir.dt.float32
F32R = mybir.dt.float32r
AF = mybir.ActivationFunctionType

NCORES = 8
B = 512
BL = B // NCORES          # 64 images per core
NI = 8                    # images per sub-batch
SUB = BL // NI            # 8 sub-batches
FR = 34 * 34              # padded frame (34x34) per image
SPAN = NI * FR            # 9248
G = 72                    # guard columns on each side (x taps need >= 71)
W = G + SPAN + G          # 9320
TAPS = [(dy, dx) for dy in (-1, 0, 1) for dx in (-1, 0, 1)]
CH = [(0, 2048), (2048, 2048), (4096, 2048), (6144, 2048), (8192, 1056)]
L = 32
S = 256


def _banks(length):
    return [(b, min(512, length - b)) for b in range(0, length, 512)]


def _frames_view(t, p0, p1, d):
    """(p1-p0, NI, 34, 34) view of tile t cols [G - d, ...), i.e. the frame
    grid shifted so (n,h,w) maps to col G + n*FR + 34*h + w - d."""
    return t[p0:p1, G - d:G - d + NI * FR].rearrange(
        "p (n h w) -> p n h w", n=NI, h=34, w=34)


def build():
    nc = bacc.Bacc(None, target_bir_lowering=False, debug=False)

    x_d = nc.declare_dram_parameter("x", [BL, 3, 32, 32], F32, isOutput=False)
    c1_d = nc.declare_dram_parameter("c1T", [32, 64], F32, isOutput=False)
    c2p_d = nc.declare_dram_parameter("c2p", [3, 128, 128], F32, isOutput=False)
    c2s_d = nc.declare_dram_parameter("c2s", [3, 64, 128], F32, isOutput=False)
    c3_d = nc.declare_dram_parameter("c3T", [9, 128, 128], F32, isOutput=False)
    sc_d = {}
    for i, cc in ((1, 64), (2, 128), (3, 128)):
        sc_d[i] = (nc.declare_dram_parameter(f"inv{i}", [cc], F32, isOutput=False),
                   nc.declare_dram_parameter(f"beta{i}", [cc], F32, isOutput=False))
    bt_d = nc.declare_dram_parameter("BT", [128, S], F32, isOutput=False)
    wt_d = nc.declare_dram_parameter("Wt", [128, 2, L], F32, isOutput=False)
    ct_d = nc.declare_dram_parameter("Ct", [2, 128, S], F32, isOutput=False)
    dt_d = nc.declare_dram_parameter("Dt", [128, S], F32, isOutput=False)
    w1_d = nc.declare_dram_parameter("w1T", [2, 128, 128], F32, isOutput=False)
    w2_d = nc.declare_dram_parameter("w2T", [128, 10], F32, isOutput=False)
    b1_d = nc.declare_dram_parameter("hb1", [128], F32, isOutput=False)
    b2_d = nc.declare_dram_parameter("hb2", [10], F32, isOutput=False)
    pb_d = nc.declare_dram_parameter("pbias", [128, 2], F32, isOutput=False)
    out1_d = nc.declare_dram_parameter("out1", [BL, 10], F32, isOutput=True)
    out2_d = nc.declare_dram_parameter("out2", [BL, S], F32, isOutput=True)

    with tile.TileContext(nc) as tc:
        import contextlib
        with contextlib.ExitStack() as ctx:
            consts = ctx.enter_context(tc.tile_pool(name="consts", bufs=1))
            big = ctx.enter_context(tc.tile_pool(name="big", bufs=1))

            # ---- weight / const tiles
            c1w = consts.tile([32, 64], F32R)
            nc.sync.dma_start(c1w[:], c1_d[:, :].bitcast(F32R))
            c2p = consts.tile([128, 3, 128], F32R)
            nc.sync.dma_start(c2p[:], c2p_d[:, :, :].rearrange("t k m -> k t m").bitcast(F32R))
            c2s = consts.tile([64, 3, 128], F32R)
            nc.sync.dma_start(c2s[:], c2s_d[:, :, :].rearrange("t k m -> k t m").bitcast(F32R))
            c3w = consts.tile([128, 9, 128], F32R)
            nc.sync.dma_start(c3w[:], c3_d[:, :, :].rearrange("t k m -> k t m").bitcast(F32R))
            sc = {}
            for i, cc in ((1, 64), (2, 128), (3, 128)):
                s_t = consts.tile([cc, 1], F32, tag=f"inv{i}")
                nc.sync.dma_start(s_t[:], sc_d[i][0][:].unsqueeze(1))
                b_t = consts.tile([cc, 1], F32, tag=f"beta{i}")
                nc.sync.dma_start(b_t[:], sc_d[i][1][:].unsqueeze(1))
                sc[i] = (s_t, b_t)
            btw = consts.tile([128, S], F32R)
            nc.sync.dma_start(btw[:], bt_d[:, :].bitcast(F32R))
            wtw = consts.tile([128, 2, L], F32)
            nc.sync.dma_start(wtw[:], wt_d[:, :, :])
            ctw = consts.tile([128, 2, S], F32)
            nc.sync.dma_start(ctw[:], ct_d[:, :, :].rearrange("k p o -> p k o"))
            dtw = consts.tile([128, S], F32)
            nc.sync.dma_start(dtw[:], dt_d[:, :])
            w1w = consts.tile([128, 2, 128], F32)
            nc.sync.dma_start(w1w[:], w1_d[:, :, :].rearrange("m p o -> p m o"))
            w2w = consts.tile([128, 10], F32)
            nc.sync.dma_start(w2w[:], w2_d[:, :])
            b1w = consts.tile([128, 1], F32)
            nc.sync.dma_start(b1w[:], b1_d[:].unsqueeze(1))
            b2w = consts.tile([16, 1], F32)
            nc.sync.dma_start(b2w[0:10, :], b2_d[:].unsqueeze(1))
            pbw = consts.tile([128, 2], F32)
            nc.sync.dma_start(pbw[:], pb_d[:, :])
            ident = consts.tile([128, 128], F32)
            make_identity(nc, ident)

            zc = consts.tile([128, 1], F32R)
            nc.vector.memset(zc[:].bitcast(F32), 0.0)

            # ---- big tiles
            x_st = big.tile([32, W], F32R)     # 9 stacked shifted taps of x
            fm1 = big.tile([128, W], F32R)     # 0-63: conv1 out; 64-127: +1 col
            fm2 = big.tile([128, W], F32R)
            fm3 = big.tile([128, W], F32R)
            u = big.tile([128, BL, L], F32R)   # width-sums, all 64 images

            # one-time zero init (guards + rings start zero; DMAs only write
            # interiors; rings re-zeroed each sub-batch after ACT drains)
            nc.vector.tensor_copy(x_st[:], zc[0:32, :].broadcast_to((32, W)))
            nc.vector.tensor_copy(fm1[:], zc[:].broadcast_to((128, W)))
            nc.gpsimd.tensor_copy(fm2[:], zc[:].broadcast_to((128, W)))
            nc.gpsimd.tensor_copy(fm3[:], zc[:].broadcast_to((128, W)))

            def rings(t, p1, engine):
                for j in range(NI):
                    F0 = G + j * FR
                    engine.tensor_copy(t[0:p1, F0:F0 + 35],
                                       zc[0:p1, :].broadcast_to((p1, 35)))
                    rb = t[0:p1, F0 + 67:F0 + 67 + 31 * 34].rearrange(
                        "p (a b) -> p a b", b=34)[:, :, 0:2]
                    engine.tensor_copy(rb, zc[0:p1, :].broadcast_to((p1, 31, 2)))
                    engine.tensor_copy(t[0:p1, F0 + 1121:F0 + 1156],
                                       zc[0:p1, :].broadcast_to((p1, 35)))

            with tc.tile_pool(name="cps", bufs=2, space="PSUM") as cps:
                for k in range(SUB):
                    b0 = k * NI
                    # ---- stage x: 9 shifted tap replicas (DMA APs max 3 dims
                    # -> one DMA per tap per image)
                    for t, (dy, dx) in enumerate(TAPS):
                        d = 34 * dy + dx
                        for n in range(NI):
                            base = G + 35 - d + n * FR
                            dst = x_st[3 * t:3 * t + 3, base:base + 32 * 34].rearrange(
                                "p (h w) -> p h w", w=34)[:, :, 0:32]
                            src = x_d[b0 + n:b0 + n + 1, :, :, :].rearrange(
                                "one c h w -> c (one h) w").bitcast(F32R)
                            nc.sync.dma_start(out=dst, in_=src)

                    # ---- conv1
                    for (c0, ln) in CH:
                        pt = cps.tile([128, 2048], F32, tag="cps")
                        nb = _banks(ln)
                        for bi, (bo, bl) in enumerate(nb):
                            nc.tensor.matmul(
                                pt[0:64, bo:bo + bl], c1w[0:27, :],
                                x_st[0:27, G + c0 + bo:G + c0 + bo + bl],
                                start=True, stop=True)
                        nc.scalar.activation(
                            fm1[0:64, G + c0:G + c0 + ln], pt[0:64, 0:ln],
                            AF.Relu, bias=sc[1][1][:], scale=sc[1][0][:])
                    rings(fm1, 64, nc.vector)
                    for (c0, ln) in CH:
                        nc.sync.dma_start(
                            out=fm1[64:128, G + c0:G + c0 + ln],
                            in_=fm1[0:64, G + c0 + 1:G + c0 + ln + 1])

                    # ---- conv2: 3 singles (dy,+1) then 3 pairs {(dy,-1),(dy,0)}
                    for (c0, ln) in CH:
                        pt = cps.tile([128, 2048], F32, tag="cps")
                        for (bo, bl) in _banks(ln):
                            base = G + c0 + bo
                            for i, dy in enumerate((-1, 0, 1)):
                                d = 34 * dy + 1
                                nc.tensor.matmul(
                                    pt[:, bo:bo + bl], c2s[:, i, :],
                                    fm1[0:64, base + d:base + d + bl],
                                    start=(i == 0), stop=False)
                            for i, dy in enumerate((-1, 0, 1)):
                                d = 34 * dy - 1
                                nc.tensor.matmul(
                                    pt[:, bo:bo + bl], c2p[:, i, :],
                                    fm1[:, base + d:base + d + bl],
                                    start=False, stop=(i == 2))
                        nc.scalar.activation(
                            fm2[:, G + c0:G + c0 + ln], pt[:, 0:ln],
                            AF.Relu, bias=sc[2][1][:], scale=sc[2][0][:])
                    rings(fm2, 128, nc.gpsimd)

                    # ---- conv3
                    for (c0, ln) in CH:
                        pt = cps.tile([128, 2048], F32, tag="cps")
                        for (bo, bl) in _banks(ln):
                            base = G + c0 + bo
                            for t, (dy, dx) in enumerate(TAPS):
                                d = 34 * dy + dx
                                nc.tensor.matmul(
                                    pt[:, bo:bo + bl], c3w[:, t, :],
                                    fm2[:, base + d:base + d + bl],
                                    start=(t == 0), stop=(t == 8))
                        nc.scalar.activation(
                            fm3[:, G + c0:G + c0 + ln], pt[:, 0:ln],
                            AF.Relu, bias=sc[3][1][:], scale=sc[3][0][:])

                    # ---- width sums -> u[:, b0:b0+NI, :]
                    iv = _frames_view(fm3, 0, 128, 0)[:, :, 1:33, 1:33]
                    with nc.allow_low_precision(reason="f32r mantissa rounding only"):
                        nc.vector.tensor_reduce(
                            u[:, b0:b0 + NI, :], iv,
                            axis=mybir.AxisListType.X, op=mybir.AluOpType.add)

                # ---- SSM: Bu = B^T.T @ u  (2 s-tiles x 4 chunks of 512)
                uf = u[:].rearrange("p a b -> p (a b)")
                bu = []
                for m in range(2):
                    pm = cps.tile([128, 2048], F32, tag="cps")
                    for j in range(4):
                        nc.tensor.matmul(
                            pm[:, 512 * j:512 * (j + 1)],
                            btw[:, 128 * m:128 * (m + 1)],
                            uf[:, 512 * j:512 * (j + 1)],
                            start=True, stop=True)
                    bu.append(pm)
                # sx_m[s,b] = sum_tau W[s,tau] * Bu[s,b,tau]
                sx = []
                for m in range(2):
                    tmp = big.tile([128, BL, L], F32, tag="tmp")
                    nc.vector.tensor_tensor(
                        tmp[:], bu[m][:].rearrange("p (a b) -> p a b", b=L),
                        wtw[:, m:m + 1, :].broadcast_to((128, BL, L)),
                        op=mybir.AluOpType.mult)
                    sxm = big.tile([128, BL], F32, tag=f"sx{m}")
                    nc.vector.tensor_reduce(
                        sxm[:], tmp[:], axis=mybir.AxisListType.X,
                        op=mybir.AluOpType.add)
                    sx.append(sxm)
                ub = big.tile([128, BL], F32)
                nc.vector.tensor_reduce(
                    ub[:], u[:], axis=mybir.AxisListType.X, op=mybir.AluOpType.add)

            with tc.tile_pool(name="tail", bufs=1, space="PSUM") as tps:
                # pooled[o,b] = Ct.T@sx0 + ... + Dt.T@ub   (+ h0 bias via ACT)
                pooled_s = []
                o2s = big.tile([64, S], F32)
                for m in range(2):
                    pp = tps.tile([128, BL], F32, tag=f"pl{m}")
                    ops = [(ctw[:, 0, 128 * m:128 * (m + 1)], sx[0]),
                           (ctw[:, 1, 128 * m:128 * (m + 1)], sx[1]),
                           (dtw[:, 128 * m:128 * (m + 1)], ub)]
                    for i, (lt, rt) in enumerate(ops):
                        nc.tensor.matmul(pp[:], lt, rt[:],
                                         start=(i == 0), stop=(i == 2))
                    ps_t = big.tile([128, BL], F32, tag=f"pooled{m}")
                    nc.scalar.activation(ps_t[:], pp[:], AF.Identity,
                                         bias=pbw[:, m:m + 1], scale=1.0)
                    pooled_s.append(ps_t)
                    # transpose to (b, o) for the activations output
                    ptr = tps.tile([64, 128], F32, tag="ptr", bufs=2)
                    nc.tensor.transpose(ptr[:], ps_t[:], ident[:])
                    nc.vector.tensor_copy(o2s[:, 128 * m:128 * (m + 1)], ptr[:])
                nc.sync.dma_start(out2_d[:, :], o2s[:])

                # head
                hp = tps.tile([128, BL], F32, tag="hp")
                for m in range(2):
                    nc.tensor.matmul(hp[:], w1w[:, m, :], pooled_s[m][:],
                                     start=(m == 0), stop=(m == 1))
                hs = big.tile([128, BL], F32)
                nc.scalar.activation(hs[:], hp[:], AF.Relu, bias=b1w[:], scale=1.0)
                lp = tps.tile([16, BL], F32, tag="lp")
                nc.tensor.matmul(lp[0:10, :], w2w[:], hs[:], start=True, stop=True)
                ls = big.tile([16, BL], F32)
                nc.scalar.activation(ls[0:10, :], lp[0:10, :], AF.Identity,
                                     bias=b2w[0:10, :], scale=1.0)
                lt = tps.tile([64, 16], F32, tag="lt")
                nc.tensor.transpose(lt[:, 0:10], ls[0:10, :], ident[0:10, 0:10])
                o1s = big.tile([64, 16], F32)
                nc.vector.tensor_copy(o1s[:, 0:10], lt[:, 0:10])
                nc.sync.dma_start(out1_d[:, :], o1s[:, 0:10])

    nc.finalize()
    return nc


def prep_in_maps(inputs):
    f32 = np.float32
    x = np.ascontiguousarray(inputs["x"], dtype=f32)

    c1 = np.asarray(inputs["conv1_w"], dtype=f32)   # (64,3,3,3)
    c1T = np.zeros((32, 64), f32)
    for t, (dy, dx) in enumerate(TAPS):
        c1T[3 * t:3 * t + 3, :] = c1[:, :, dy + 1, dx + 1].T
    c2 = np.asarray(inputs["conv2_w"], dtype=f32)   # (128,64,3,3)
    c2p = np.zeros((3, 128, 128), f32)
    c2s = np.zeros((3, 64, 128), f32)
    for i, dy in enumerate((-1, 0, 1)):
        c2p[i, 0:64, :] = c2[:, :, dy + 1, 0].T     # tap (dy,-1) on lower
        c2p[i, 64:128, :] = c2[:, :, dy + 1, 1].T   # tap (dy, 0) via +1 copy
        c2s[i, :, :] = c2[:, :, dy + 1, 2].T        # tap (dy,+1)
    c3 = np.asarray(inputs["conv3_w"], dtype=f32)
    c3T = np.zeros((9, 128, 128), f32)
    for t, (dy, dx) in enumerate(TAPS):
        c3T[t] = c3[:, :, dy + 1, dx + 1].T

    scb = {}
    for i in (1, 2, 3):
        g = np.asarray(inputs[f"bn{i}_g"], f32)
        b = np.asarray(inputs[f"bn{i}_b"], f32)
        m = np.asarray(inputs[f"bn{i}_m"], f32)
        v = np.asarray(inputs[f"bn{i}_v"], f32)
        inv = g / np.sqrt(v + np.float32(1e-5))
        scb[i] = (inv.astype(f32), (b - m * inv).astype(f32))

    A = -np.log1p(np.exp(np.asarray(inputs["ssm_A"], np.float64)))
    # w_tau = sum_{k=0}^{L-1-tau} A^k ; scaled by 1/(32*L) for width+time means
    wts = np.stack([(1.0 - A ** (L - t)) / (1.0 - A) for t in range(L)], 1)  # (S,L)
    Wt = (wts / (32.0 * L)).astype(f32).reshape(2, 128, L).transpose(1, 0, 2)
    Wt = np.ascontiguousarray(Wt)  # (128, 2, L): [p, m, tau] = s=128m+p
    BT = np.ascontiguousarray(np.asarray(inputs["ssm_B"], f32).T)  # (128,256)
    Cm = np.asarray(inputs["ssm_C"], f32)
    Ct = np.ascontiguousarray(Cm.T.reshape(2, 128, S))
    Dt = np.ascontiguousarray((np.asarray(inputs["ssm_D"], np.float64).T / (32.0 * L)).astype(f32))
    # h0 contribution: pooled += C @ (geo * h0) / L,  geo = sum_{t=1..L} A^t
    h0 = np.asarray(inputs["ssm_h0"], np.float64)
    geo = A * (1.0 - A ** L) / (1.0 - A)
    pbias = ((Cm.astype(np.float64) @ (geo * h0)) / L).astype(f32).reshape(2, 128).T
    pbias = np.ascontiguousarray(pbias)  # (128, 2)

    w1T = np.ascontiguousarray(np.asarray(inputs["head_w1"], f32).T.reshape(2, 128, 128))
    w2T = np.ascontiguousarray(np.asarray(inputs["head_w2"], f32).T)
    hb1 = np.asarray(inputs["head_b1"], f32)
    hb2 = np.asarray(inputs["head_b2"], f32)

    shared = dict(c1T=c1T, c2p=c2p, c2s=c2s, c3T=c3T,
                  inv1=scb[1][0], beta1=scb[1][1],
                  inv2=scb[2][0], beta2=scb[2][1],
                  inv3=scb[3][0], beta3=scb[3][1],
                  BT=BT, Wt=Wt, Ct=Ct, Dt=Dt, w1T=w1T, w2T=w2T,
                  hb1=hb1, hb2=hb2, pbias=pbias)
    in_maps = []
    for i in range(NCORES):
        m = dict(shared)
        m["x"] = np.ascontiguousarray(x[i * BL:(i + 1) * BL])
        in_maps.append(m)
    return in_maps


_NC_CACHE = []


def kernel(**inputs):
    if not _NC_CACHE:
        _NC_CACHE.append(build())
    nc = _NC_CACHE[0]
    in_maps = prep_in_maps(inputs)
    res = run_bass_kernel_spmd(nc, in_maps, core_ids=list(range(NCORES)))
    out = np.concatenate([res.results[i]["out1"] for i in range(NCORES)], axis=0)
    act = np.concatenate([res.results[i]["out2"] for i in range(NCORES)], axis=0)
    return out.astype(np.float32), act.astype(np.float32)


# revision 4
# speedup vs baseline: 1.0150x; 1.0150x over previous
"""Trainium2 Bass kernel for nn_CIFAR_SSM_Classifier.

Data-parallel over 8 NeuronCores: each core processes 64 of the 512 images.

Per-core pipeline (SBUF-resident, bf16 matmuls on the PE, fp32 accumulate):
  conv1 (3->64, 3x3)  : 9 taps stacked on K (27 rows), 1 matmul per 512-col bank
  conv2 (64->128,3x3) : 3 col-adjacent tap pairs stacked to K=128 (via a
                        shifted copy of fm1 in partitions 64-127) + 3 singles
  conv3 (128->128,3x3): 9 matmuls of K=128 per bank
  BN+ReLU fused into the PSUM->SBUF drain on the Scalar engine.
  Feature maps use a zero-padded 34x34 frame layout per image so all 9 conv
  taps are plain column offsets of one SBUF tile.
  width-mean -> SSM: the L=32 diagonal scan is unrolled algebraically:
    sum_t x_t = sum_tau w_tau (.) (B u_tau),  w_tau = sum_{k<=L-1-tau} A^k
  so the whole SSM+head collapses into a few small matmuls + one weighted
  reduction. BN folding and the geometric weights are precomputed host-side
  (numpy) from the parameter tensors.
"""
import numpy as np
import ml_dtypes

import concourse.bass as bass
import concourse.tile as tile
from concourse import bacc, mybir
from concourse.bass_utils import run_bass_kernel_spmd
from concourse.masks import make_identity

F32 = mybir.dt.float32
BF16 = mybir.dt.bfloat16
AF = mybir.ActivationFunctionType

NCORES = 8
B = 512
BL = B // NCORES          # 64 images per core
NI = 16                   # images per sub-batch
SUB = BL // NI            # 4 sub-batches
FR = 34 * 34              # padded frame (34x34) per image
SPAN = NI * FR            # 18496
G = 72                    # guard columns on each side (x taps need >= 71)
W = G + SPAN + G
TAPS = [(dy, dx) for dy in (-1, 0, 1) for dx in (-1, 0, 1)]
CH = [(c, min(2048, SPAN - c)) for c in range(0, SPAN, 2048)]
L = 32
S = 256


def _banks(length):
    return [(b, min(512, length - b)) for b in range(0, length, 512)]


def build():
    nc = bacc.Bacc(None, target_bir_lowering=False, debug=False)

    x_d = nc.declare_dram_parameter("x", [BL, 3, 32, 32], BF16, isOutput=False)
    c1_d = nc.declare_dram_parameter("c1T", [32, 64], BF16, isOutput=False)
    c2p_d = nc.declare_dram_parameter("c2p", [3, 128, 128], BF16, isOutput=False)
    c2s_d = nc.declare_dram_parameter("c2s", [3, 64, 128], BF16, isOutput=False)
    c3_d = nc.declare_dram_parameter("c3T", [9, 128, 128], BF16, isOutput=False)
    sc_d = {}
    for i, cc in ((1, 64), (2, 128), (3, 128)):
        sc_d[i] = (nc.declare_dram_parameter(f"inv{i}", [cc], F32, isOutput=False),
                   nc.declare_dram_parameter(f"beta{i}", [cc], F32, isOutput=False))
    bt_d = nc.declare_dram_parameter("BT", [128, S], BF16, isOutput=False)
    wt_d = nc.declare_dram_parameter("Wt", [128, 2, L], F32, isOutput=False)
    ct_d = nc.declare_dram_parameter("Ct", [2, 128, S], F32, isOutput=False)
    dt_d = nc.declare_dram_parameter("Dt", [128, S], F32, isOutput=False)
    w1_d = nc.declare_dram_parameter("w1T", [2, 128, 128], F32, isOutput=False)
    w2_d = nc.declare_dram_parameter("w2T", [128, 10], F32, isOutput=False)
    b1_d = nc.declare_dram_parameter("hb1", [128], F32, isOutput=False)
    b2_d = nc.declare_dram_parameter("hb2", [10], F32, isOutput=False)
    pb_d = nc.declare_dram_parameter("pbias", [128, 2], F32, isOutput=False)
    out1_d = nc.declare_dram_parameter("out1", [BL, 10], F32, isOutput=True)
    out2_d = nc.declare_dram_parameter("out2", [BL, S], F32, isOutput=True)

    with tile.TileContext(nc) as tc:
        import contextlib
        with contextlib.ExitStack() as ctx:
            consts = ctx.enter_context(tc.tile_pool(name="consts", bufs=1))
            big = ctx.enter_context(tc.tile_pool(name="big", bufs=1))

            # ---- weight / const tiles
            c1w = consts.tile([32, 64], BF16)
            nc.sync.dma_start(c1w[:], c1_d[:, :])
            c2p = consts.tile([128, 3, 128], BF16)
            nc.sync.dma_start(c2p[:], c2p_d[:, :, :].rearrange("t k m -> k t m"))
            c2s = consts.tile([64, 3, 128], BF16)
            nc.sync.dma_start(c2s[:], c2s_d[:, :, :].rearrange("t k m -> k t m"))
            c3w = consts.tile([128, 9, 128], BF16)
            nc.sync.dma_start(c3w[:], c3_d[:, :, :].rearrange("t k m -> k t m"))
            sc = {}
            for i, cc in ((1, 64), (2, 128), (3, 128)):
                s_t = consts.tile([cc, 1], F32, tag=f"inv{i}")
                nc.sync.dma_start(s_t[:], sc_d[i][0][:].unsqueeze(1))
                b_t = consts.tile([cc, 1], F32, tag=f"beta{i}")
                nc.sync.dma_start(b_t[:], sc_d[i][1][:].unsqueeze(1))
                sc[i] = (s_t, b_t)
            btw = consts.tile([128, S], BF16)
            nc.sync.dma_start(btw[:], bt_d[:, :])
            wtw = consts.tile([128, 2, L], F32)
            nc.sync.dma_start(wtw[:], wt_d[:, :, :])
            ctw = consts.tile([128, 2, S], F32)
            nc.sync.dma_start(ctw[:], ct_d[:, :, :].rearrange("k p o -> p k o"))
            dtw = consts.tile([128, S], F32)
            nc.sync.dma_start(dtw[:], dt_d[:, :])
            w1w = consts.tile([128, 2, 128], F32)
            nc.sync.dma_start(w1w[:], w1_d[:, :, :].rearrange("m p o -> p m o"))
            w2w = consts.tile([128, 10], F32)
            nc.sync.dma_start(w2w[:], w2_d[:, :])
            b1w = consts.tile([128, 1], F32)
            nc.sync.dma_start(b1w[:], b1_d[:].unsqueeze(1))
            b2w = consts.tile([16, 1], F32)
            nc.sync.dma_start(b2w[0:10, :], b2_d[:].unsqueeze(1))
            pbw = consts.tile([128, 2], F32)
            nc.sync.dma_start(pbw[:], pb_d[:, :])
            ident = consts.tile([128, 128], F32)
            make_identity(nc, ident)

            zc = consts.tile([128, 1], BF16)
            nc.vector.memset(zc[:], 0.0)

            # ---- big tiles
            x_st = big.tile([32, W], BF16)     # 9 stacked shifted taps of x
            fm1 = big.tile([128, W], BF16)     # 0-63: conv1 out; 64-127: +1 col
            fm2 = big.tile([128, W], BF16)
            fm3 = big.tile([128, W], BF16)
            u = big.tile([128, BL, L], BF16)   # width-sums, all 64 images

            # one-time zero init (guards + rings start zero; DMAs only write
            # interiors; rings re-zeroed each sub-batch after ACT drains)
            nc.vector.tensor_copy(x_st[:], zc[0:32, :].broadcast_to((32, W)))
            nc.vector.tensor_copy(fm1[:], zc[:].broadcast_to((128, W)))
            nc.gpsimd.tensor_copy(fm2[:], zc[:].broadcast_to((128, W)))

            def rings(t, p1, engine):
                for j in range(NI):
                    F0 = G + j * FR
                    engine.tensor_copy(t[0:p1, F0:F0 + 35],
                                       zc[0:p1, :].broadcast_to((p1, 35)))
                    rb = t[0:p1, F0 + 67:F0 + 67 + 31 * 34].rearrange(
                        "p (a b) -> p a b", b=34)[:, :, 0:2]
                    engine.tensor_copy(rb, zc[0:p1, :].broadcast_to((p1, 31, 2)))
                    engine.tensor_copy(t[0:p1, F0 + 1121:F0 + 1156],
                                       zc[0:p1, :].broadcast_to((p1, 35)))

            with tc.tile_pool(name="cps", bufs=2, space="PSUM") as cps:
                for k in range(SUB):
                    b0 = k * NI
                    # ---- stage x: 9 shifted tap replicas (DMA APs max 3 dims
                    # -> one DMA per tap per image)
                    for t, (dy, dx) in enumerate(TAPS):
                        d = 34 * dy + dx
                        for n in range(NI):
                            base = G + 35 - d + n * FR
                            dst = x_st[3 * t:3 * t + 3, base:base + 32 * 34].rearrange(
                                "p (h w) -> p h w", w=34)[:, :, 0:32]
                            src = x_d[b0 + n:b0 + n + 1, :, :, :].rearrange(
                                "one c h w -> c (one h) w")
                            nc.sync.dma_start(out=dst, in_=src)

                    # ---- conv1
                    for (c0, ln) in CH:
                        pt = cps.tile([128, 2048], F32, tag="cps")
                        for (bo, bl) in _banks(ln):
                            nc.tensor.matmul(
                                pt[0:64, bo:bo + bl], c1w[0:27, :],
                                x_st[0:27, G + c0 + bo:G + c0 + bo + bl],
                                start=True, stop=True)
                        nc.scalar.activation(
                            fm1[0:64, G + c0:G + c0 + ln], pt[0:64, 0:ln],
                            AF.Relu, bias=sc[1][1][:], scale=sc[1][0][:])
                    rings(fm1, 64, nc.vector)
                    for (c0, ln) in CH:
                        nc.sync.dma_start(
                            out=fm1[64:128, G + c0:G + c0 + ln],
                            in_=fm1[0:64, G + c0 + 1:G + c0 + ln + 1])

                    # ---- conv2: 3 singles (dy,+1) then 3 pairs {(dy,-1),(dy,0)}
                    for (c0, ln) in CH:
                        pt = cps.tile([128, 2048], F32, tag="cps")
                        for (bo, bl) in _banks(ln):
                            base = G + c0 + bo
                            for i, dy in enumerate((-1, 0, 1)):
                                d = 34 * dy + 1
                                nc.tensor.matmul(
                                    pt[:, bo:bo + bl], c2s[:, i, :],
                                    fm1[0:64, base + d:base + d + bl],
                                    start=(i == 0), stop=False)
                            for i, dy in enumerate((-1, 0, 1)):
                                d = 34 * dy - 1
                                nc.tensor.matmul(
                                    pt[:, bo:bo + bl], c2p[:, i, :],
                                    fm1[:, base + d:base + d + bl],
                                    start=False, stop=(i == 2))
                        nc.scalar.activation(
                            fm2[:, G + c0:G + c0 + ln], pt[:, 0:ln],
                            AF.Relu, bias=sc[2][1][:], scale=sc[2][0][:])
                    rings(fm2, 128, nc.gpsimd)

                    # ---- conv3
                    for (c0, ln) in CH:
                        pt = cps.tile([128, 2048], F32, tag="cps")
                        for (bo, bl) in _banks(ln):
                            base = G + c0 + bo
                            for t, (dy, dx) in enumerate(TAPS):
                                d = 34 * dy + dx
                                nc.tensor.matmul(
                                    pt[:, bo:bo + bl], c3w[:, t, :],
                                    fm2[:, base + d:base + d + bl],
                                    start=(t == 0), stop=(t == 8))
                        nc.scalar.activation(
                            fm3[:, G + c0:G + c0 + ln], pt[:, 0:ln],
                            AF.Relu, bias=sc[3][1][:], scale=sc[3][0][:])

                    # ---- width sums -> u[:, b0:b0+NI, :]
                    iv = fm3[:, G:G + NI * FR].rearrange(
                        "p (n h w) -> p n h w", n=NI, h=34, w=34)[:, :, 1:33, 1:33]
                    with nc.allow_low_precision(reason="bf16 u; validated 1.7e-3"):
                        nc.vector.tensor_reduce(
                            u[:, b0:b0 + NI, :], iv,
                            axis=mybir.AxisListType.X, op=mybir.AluOpType.add)

                # ---- SSM: Bu = B^T.T @ u  (2 s-tiles x 4 chunks of 512)
                uf = u[:].rearrange("p a b -> p (a b)")
                bu = []
                for m in range(2):
                    pm = cps.tile([128, 2048], F32, tag="cps")
                    for j in range(4):
                        nc.tensor.matmul(
                            pm[:, 512 * j:512 * (j + 1)],
                            btw[:, 128 * m:128 * (m + 1)],
                            uf[:, 512 * j:512 * (j + 1)],
                            start=True, stop=True)
                    bu.append(pm)
                # sx_m[s,b] = sum_tau W[s,tau] * Bu[s,b,tau]
                sx = []
                for m in range(2):
                    tmp = big.tile([128, BL, L], F32, tag="tmp")
                    nc.vector.tensor_tensor(
                        tmp[:], bu[m][:].rearrange("p (a b) -> p a b", b=L),
                        wtw[:, m:m + 1, :].broadcast_to((128, BL, L)),
                        op=mybir.AluOpType.mult)
                    sxm = big.tile([128, BL], F32, tag=f"sx{m}")
                    nc.vector.tensor_reduce(
                        sxm[:], tmp[:], axis=mybir.AxisListType.X,
                        op=mybir.AluOpType.add)
                    sx.append(sxm)
                ub = big.tile([128, BL], F32)
                with nc.allow_low_precision(reason="sum of bf16 u in f32 out"):
                    nc.vector.tensor_reduce(
                        ub[:], u[:], axis=mybir.AxisListType.X,
                        op=mybir.AluOpType.add)

            with tc.tile_pool(name="tail", bufs=1, space="PSUM") as tps:
                # pooled[o,b] = Ct.T@sx0 + ... + Dt.T@ub   (+ h0 bias via ACT)
                pooled_s = []
                o2s = big.tile([64, S], F32)
                for m in range(2):
                    pp = tps.tile([128, BL], F32, tag=f"pl{m}")
                    ops = [(ctw[:, 0, 128 * m:128 * (m + 1)], sx[0]),
                           (ctw[:, 1, 128 * m:128 * (m + 1)], sx[1]),
                           (dtw[:, 128 * m:128 * (m + 1)], ub)]
                    for i, (lt_, rt) in enumerate(ops):
                        nc.tensor.matmul(pp[:], lt_, rt[:],
                                         start=(i == 0), stop=(i == 2))
                    ps_t = big.tile([128, BL], F32, tag=f"pooled{m}")
                    nc.scalar.activation(ps_t[:], pp[:], AF.Identity,
                                         bias=pbw[:, m:m + 1], scale=1.0)
                    pooled_s.append(ps_t)
                    # transpose to (b, o) for the activations output
                    ptr = tps.tile([64, 128], F32, tag="ptr", bufs=2)
                    nc.tensor.transpose(ptr[:], ps_t[:], ident[:])
                    nc.vector.tensor_copy(o2s[:, 128 * m:128 * (m + 1)], ptr[:])
                nc.sync.dma_start(out2_d[:, :], o2s[:])

                # head
                hp = tps.tile([128, BL], F32, tag="hp")
                for m in range(2):
                    nc.tensor.matmul(hp[:], w1w[:, m, :], pooled_s[m][:],
                                     start=(m == 0), stop=(m == 1))
                hs = big.tile([128, BL], F32)
                nc.scalar.activation(hs[:], hp[:], AF.Relu, bias=b1w[:], scale=1.0)
                lp = tps.tile([16, BL], F32, tag="lp")
                nc.tensor.matmul(lp[0:10, :], w2w[:], hs[:], start=True, stop=True)
                ls = big.tile([16, BL], F32)
                nc.scalar.activation(ls[0:10, :], lp[0:10, :], AF.Identity,
                                     bias=b2w[0:10, :], scale=1.0)
                lt = tps.tile([64, 16], F32, tag="lt")
                nc.tensor.transpose(lt[:, 0:10], ls[0:10, :], ident[0:10, 0:10])
                o1s = big.tile([64, 16], F32)
                nc.vector.tensor_copy(o1s[:, 0:10], lt[:, 0:10])
                nc.sync.dma_start(out1_d[:, :], o1s[:, 0:10])

    nc.finalize()
    return nc


def prep_in_maps(inputs):
    f32 = np.float32
    bf = ml_dtypes.bfloat16

    c1 = np.asarray(inputs["conv1_w"], dtype=f32)   # (64,3,3,3)
    c1T = np.zeros((32, 64), f32)
    for t, (dy, dx) in enumerate(TAPS):
        c1T[3 * t:3 * t + 3, :] = c1[:, :, dy + 1, dx + 1].T
    c2 = np.asarray(inputs["conv2_w"], dtype=f32)   # (128,64,3,3)
    c2p = np.zeros((3, 128, 128), f32)
    c2s = np.zeros((3, 64, 128), f32)
    for i, dy in enumerate((-1, 0, 1)):
        c2p[i, 0:64, :] = c2[:, :, dy + 1, 0].T     # tap (dy,-1) on lower
        c2p[i, 64:128, :] = c2[:, :, dy + 1, 1].T   # tap (dy, 0) via +1 copy
        c2s[i, :, :] = c2[:, :, dy + 1, 2].T        # tap (dy,+1)
    c3 = np.asarray(inputs["conv3_w"], dtype=f32)
    c3T = np.zeros((9, 128, 128), f32)
    for t, (dy, dx) in enumerate(TAPS):
        c3T[t] = c3[:, :, dy + 1, dx + 1].T

    scb = {}
    for i in (1, 2, 3):
        g = np.asarray(inputs[f"bn{i}_g"], f32)
        b = np.asarray(inputs[f"bn{i}_b"], f32)
        m = np.asarray(inputs[f"bn{i}_m"], f32)
        v = np.asarray(inputs[f"bn{i}_v"], f32)
        inv = g / np.sqrt(v + np.float32(1e-5))
        scb[i] = (inv.astype(f32), (b - m * inv).astype(f32))

    A = -np.log1p(np.exp(np.asarray(inputs["ssm_A"], np.float64)))
    # w_tau = sum_{k=0}^{L-1-tau} A^k ; scaled by 1/(32*L) for width+time means
    wts = np.stack([(1.0 - A ** (L - t)) / (1.0 - A) for t in range(L)], 1)  # (S,L)
    Wt = (wts / (32.0 * L)).astype(f32).reshape(2, 128, L).transpose(1, 0, 2)
    Wt = np.ascontiguousarray(Wt)  # (128, 2, L): [p, m, tau] = s=128m+p
    BT = np.ascontiguousarray(np.asarray(inputs["ssm_B"], f32).T)  # (128,256)
    Cm = np.asarray(inputs["ssm_C"], f32)
    Ct = np.ascontiguousarray(Cm.T.reshape(2, 128, S))
    Dt = np.ascontiguousarray((np.asarray(inputs["ssm_D"], np.float64).T / (32.0 * L)).astype(f32))
    # h0 contribution: pooled += C @ (geo * h0) / L,  geo = sum_{t=1..L} A^t
    h0 = np.asarray(inputs["ssm_h0"], np.float64)
    geo = A * (1.0 - A ** L) / (1.0 - A)
    pbias = ((Cm.astype(np.float64) @ (geo * h0)) / L).astype(f32).reshape(2, 128).T
    pbias = np.ascontiguousarray(pbias)  # (128, 2)

    w1T = np.ascontiguousarray(np.asarray(inputs["head_w1"], f32).T.reshape(2, 128, 128))
    w2T = np.ascontiguousarray(np.asarray(inputs["head_w2"], f32).T)
    hb1 = np.asarray(inputs["head_b1"], f32)
    hb2 = np.asarray(inputs["head_b2"], f32)

    x = np.asarray(inputs["x"], f32).astype(bf)
    shared = dict(c1T=c1T.astype(bf), c2p=c2p.astype(bf), c2s=c2s.astype(bf),
                  c3T=c3T.astype(bf),
                  inv1=scb[1][0], beta1=scb[1][1],
                  inv2=scb[2][0], beta2=scb[2][1],
                  inv3=scb[3][0], beta3=scb[3][1],
                  BT=BT.astype(bf), Wt=Wt, Ct=Ct, Dt=Dt, w1T=w1T, w2T=w2T,
                  hb1=hb1, hb2=hb2, pbias=pbias)
    in_maps = []
    for i in range(NCORES):
        m = dict(shared)
        m["x"] = np.ascontiguousarray(x[i * BL:(i + 1) * BL])
        in_maps.append(m)
    return in_maps


_NC_CACHE = []


def kernel(**inputs):
    if not _NC_CACHE:
        _NC_CACHE.append(build())
    nc = _NC_CACHE[0]
    in_maps = prep_in_maps(inputs)
    res = run_bass_kernel_spmd(nc, in_maps, core_ids=list(range(NCORES)))
    out = np.concatenate([res.results[i]["out1"] for i in range(NCORES)], axis=0)
    act = np.concatenate([res.results[i]["out2"] for i in range(NCORES)], axis=0)
    return out.astype(np.float32), act.astype(np.float32)


# revision 6
# speedup vs baseline: 1.2382x; 1.2199x over previous
"""Trainium2 Bass kernel for nn_CIFAR_SSM_Classifier.

Data-parallel over 8 NeuronCores: each core processes 64 of the 512 images.

Per-core pipeline (SBUF-resident, bf16 matmuls on the PE, fp32 accumulate).
All conv matmuls use the full K=128 contraction rows (partial-K matmuls let
the PE activity monitor drop the clock to 1.2 GHz):
  conv1 (3->64)   : 9 taps x 3ch stacked on K (27 rows, zero-padded to 128);
                    the tap-shifted/zero-padded input layout is staged host-side
                    and DMA'd once per sub-batch.
  conv2 (64->128) : 5 K=128 matmuls per 512-col bank: 3 pairs {(dy,-1),(dy,0)}
                    via fm1 upper half = fm1 shifted +1 col; 1 pair
                    {(-1,+1),(0,+1)} via scratch tile (lower=fm1, upper=fm1
                    shifted +34); 1 single (1,+1) with zero-padded weights.
  conv3 (128->128): 9 K=128 matmuls per bank.
  BN+ReLU fused into the PSUM->SBUF drain on the Scalar engine.
  Feature maps use a zero-padded 34x34 frame layout per image so all taps are
  plain column offsets of one SBUF tile.
  width-mean -> SSM: the L=32 diagonal scan is unrolled algebraically:
    sum_t x_t = sum_tau w_tau (.) (B u_tau),  w_tau = sum_{k<=L-1-tau} A^k
  so the SSM+head collapses into a few small matmuls + one weighted reduction.
  BN folding and the geometric weights are precomputed host-side (numpy).
"""
import numpy as np
import ml_dtypes

import concourse.bass as bass
import concourse.tile as tile
from concourse import bacc, mybir
from concourse.bass_utils import run_bass_kernel_spmd
from concourse.masks import make_identity

F32 = mybir.dt.float32
BF16 = mybir.dt.bfloat16
AF = mybir.ActivationFunctionType

NCORES = 8
B = 512
BL = B // NCORES          # 64 images per core
NI = 16                   # images per sub-batch
SUB = BL // NI            # 4 sub-batches
FR = 34 * 34              # padded frame (34x34) per image
SPAN = NI * FR            # 18496
G = 72                    # guard columns on each side
W = G + SPAN + G
TAPS = [(dy, dx) for dy in (-1, 0, 1) for dx in (-1, 0, 1)]
CH = [(c, min(2048, SPAN - c)) for c in range(0, SPAN, 2048)]
L = 32
S = 256


def _banks(length):
    return [(b, min(512, length - b)) for b in range(0, length, 512)]


def build():
    nc = bacc.Bacc(None, target_bir_lowering=False, debug=False)

    x_d = nc.declare_dram_parameter("xst", [SUB, 32, W], BF16, isOutput=False)
    c1_d = nc.declare_dram_parameter("c1T", [128, 64], BF16, isOutput=False)
    c2q_d = nc.declare_dram_parameter("c2q", [5, 128, 128], BF16, isOutput=False)
    c3_d = nc.declare_dram_parameter("c3T", [9, 128, 128], BF16, isOutput=False)
    sc_d = {}
    for i, cc in ((1, 64), (2, 128), (3, 128)):
        sc_d[i] = (nc.declare_dram_parameter(f"inv{i}", [cc], F32, isOutput=False),
                   nc.declare_dram_parameter(f"beta{i}", [cc], F32, isOutput=False))
    bt_d = nc.declare_dram_parameter("BT", [128, S], BF16, isOutput=False)
    wt_d = nc.declare_dram_parameter("Wt", [128, 2, L], F32, isOutput=False)
    ct_d = nc.declare_dram_parameter("Ct", [2, 128, S], F32, isOutput=False)
    dt_d = nc.declare_dram_parameter("Dt", [128, S], F32, isOutput=False)
    w1_d = nc.declare_dram_parameter("w1T", [2, 128, 128], F32, isOutput=False)
    w2_d = nc.declare_dram_parameter("w2T", [128, 10], F32, isOutput=False)
    b1_d = nc.declare_dram_parameter("hb1", [128], F32, isOutput=False)
    b2_d = nc.declare_dram_parameter("hb2", [10], F32, isOutput=False)
    pb_d = nc.declare_dram_parameter("pbias", [128, 2], F32, isOutput=False)
    out1_d = nc.declare_dram_parameter("out1", [BL, 10], F32, isOutput=True)
    out2_d = nc.declare_dram_parameter("out2", [BL, S], F32, isOutput=True)

    with tile.TileContext(nc) as tc:
        import contextlib
        with contextlib.ExitStack() as ctx:
            consts = ctx.enter_context(tc.tile_pool(name="consts", bufs=1))
            big = ctx.enter_context(tc.tile_pool(name="big", bufs=1))

            # ---- weight / const tiles
            c1w = consts.tile([128, 64], BF16)
            nc.sync.dma_start(c1w[:], c1_d[:, :])
            c2w = consts.tile([128, 5, 128], BF16)
            nc.sync.dma_start(c2w[:], c2q_d[:, :, :].rearrange("t k m -> k t m"))
            c3w = consts.tile([128, 9, 128], BF16)
            nc.sync.dma_start(c3w[:], c3_d[:, :, :].rearrange("t k m -> k t m"))
            sc = {}
            for i, cc in ((1, 64), (2, 128), (3, 128)):
                s_t = consts.tile([cc, 1], F32, tag=f"inv{i}")
                nc.sync.dma_start(s_t[:], sc_d[i][0][:].unsqueeze(1))
                b_t = consts.tile([cc, 1], F32, tag=f"beta{i}")
                nc.sync.dma_start(b_t[:], sc_d[i][1][:].unsqueeze(1))
                sc[i] = (s_t, b_t)
            btw = consts.tile([128, S], BF16)
            nc.sync.dma_start(btw[:], bt_d[:, :])
            wtw = consts.tile([128, 2, L], F32)
            nc.sync.dma_start(wtw[:], wt_d[:, :, :])
            ctw = consts.tile([128, 2, S], F32)
            nc.sync.dma_start(ctw[:], ct_d[:, :, :].rearrange("k p o -> p k o"))
            dtw = consts.tile([128, S], F32)
            nc.sync.dma_start(dtw[:], dt_d[:, :])
            w1w = consts.tile([128, 2, 128], F32)
            nc.sync.dma_start(w1w[:], w1_d[:, :, :].rearrange("m p o -> p m o"))
            w2w = consts.tile([128, 10], F32)
            nc.sync.dma_start(w2w[:], w2_d[:, :])
            b1w = consts.tile([128, 1], F32)
            nc.sync.dma_start(b1w[:], b1_d[:].unsqueeze(1))
            b2w = consts.tile([16, 1], F32)
            nc.sync.dma_start(b2w[0:10, :], b2_d[:].unsqueeze(1))
            pbw = consts.tile([128, 2], F32)
            nc.sync.dma_start(pbw[:], pb_d[:, :])
            ident = consts.tile([128, 128], F32)
            make_identity(nc, ident)

            zc = consts.tile([128, 1], BF16)
            nc.vector.memset(zc[:], 0.0)

            # ---- big tiles
            x_st = big.tile([128, W], BF16)    # 0-26: staged taps, 27-127 zero
            fm1 = big.tile([128, W], BF16)     # 0-63: conv1 out; 64-127: +1 col
            fm2 = big.tile([128, W], BF16)
            scr = big.tile([128, W], BF16)     # conv2: fm1b (+0/+34); conv3: fm3
            u = big.tile([128, BL, L], BF16)   # width-sums, all 64 images

            # one-time zero init (guards + rings start zero; rings re-zeroed
            # each sub-batch after the full-span ACT drains)
            nc.vector.tensor_copy(x_st[:], zc[:].broadcast_to((128, W)))
            nc.vector.tensor_copy(fm1[:], zc[:].broadcast_to((128, W)))
            nc.gpsimd.tensor_copy(fm2[:], zc[:].broadcast_to((128, W)))
            nc.gpsimd.tensor_copy(scr[:], zc[:].broadcast_to((128, W)))

            def rings(t, p1, engine):
                for j in range(NI):
                    F0 = G + j * FR
                    engine.tensor_copy(t[0:p1, F0:F0 + 35],
                                       zc[0:p1, :].broadcast_to((p1, 35)))
                    rb = t[0:p1, F0 + 67:F0 + 67 + 31 * 34].rearrange(
                        "p (a b) -> p a b", b=34)[:, :, 0:2]
                    engine.tensor_copy(rb, zc[0:p1, :].broadcast_to((p1, 31, 2)))
                    engine.tensor_copy(t[0:p1, F0 + 1121:F0 + 1156],
                                       zc[0:p1, :].broadcast_to((p1, 35)))

            with tc.tile_pool(name="cps", bufs=2, space="PSUM") as cps:
                for k in range(SUB):
                    b0 = k * NI
                    # ---- stage x: one DMA (host pre-builds tap layout)
                    nc.sync.dma_start(out=x_st[0:32, :], in_=x_d[k, :, :])

                    # ---- conv1 (K padded to 128; rows 27-127 of lhsT are 0)
                    for (c0, ln) in CH:
                        pt = cps.tile([128, 2048], F32, tag="cps")
                        for (bo, bl) in _banks(ln):
                            nc.tensor.matmul(
                                pt[0:64, bo:bo + bl], c1w[:],
                                x_st[:, G + c0 + bo:G + c0 + bo + bl],
                                start=True, stop=True)
                        nc.scalar.activation(
                            fm1[0:64, G + c0:G + c0 + ln], pt[0:64, 0:ln],
                            AF.Relu, bias=sc[1][1][:], scale=sc[1][0][:])
                    rings(fm1, 64, nc.vector)
                    # staging for conv2: fm1 upper = fm1+1; scr = [fm1; fm1+34]
                    for ci, (c0, ln) in enumerate(CH):
                        a = G + c0
                        nc.sync.dma_start(out=fm1[64:128, a:a + ln],
                                          in_=fm1[0:64, a + 1:a + ln + 1])
                        nc.gpsimd.dma_start(out=scr[0:64, a:a + ln],
                                            in_=fm1[0:64, a:a + ln])
                        nc.scalar.dma_start(out=scr[64:128, a:a + ln],
                                            in_=fm1[0:64, a + 34:a + ln + 34])

                    # ---- conv2: 5 x K=128 per bank
                    for (c0, ln) in CH:
                        pt = cps.tile([128, 2048], F32, tag="cps")
                        for (bo, bl) in _banks(ln):
                            base = G + c0 + bo
                            mms = [(0, fm1, -35), (1, fm1, -1), (2, fm1, 33),
                                   (3, scr, -33), (4, scr, 35)]
                            for qi, (q, src, d) in enumerate(mms):
                                nc.tensor.matmul(
                                    pt[:, bo:bo + bl], c2w[:, q, :],
                                    src[:, base + d:base + d + bl],
                                    start=(qi == 0), stop=(qi == 4))
                        nc.scalar.activation(
                            fm2[:, G + c0:G + c0 + ln], pt[:, 0:ln],
                            AF.Relu, bias=sc[2][1][:], scale=sc[2][0][:])
                    rings(fm2, 128, nc.gpsimd)

                    # ---- conv3 (fm3 lives in scr; conv2 reads of scr are done)
                    for (c0, ln) in CH:
                        pt = cps.tile([128, 2048], F32, tag="cps")
                        for (bo, bl) in _banks(ln):
                            base = G + c0 + bo
                            for t, (dy, dx) in enumerate(TAPS):
                                d = 34 * dy + dx
                                nc.tensor.matmul(
                                    pt[:, bo:bo + bl], c3w[:, t, :],
                                    fm2[:, base + d:base + d + bl],
                                    start=(t == 0), stop=(t == 8))
                        nc.scalar.activation(
                            scr[:, G + c0:G + c0 + ln], pt[:, 0:ln],
                            AF.Relu, bias=sc[3][1][:], scale=sc[3][0][:])

                    # ---- width sums -> u[:, b0:b0+NI, :]
                    iv = scr[:, G:G + NI * FR].rearrange(
                        "p (n h w) -> p n h w", n=NI, h=34, w=34)[:, :, 1:33, 1:33]
                    with nc.allow_low_precision(reason="bf16 u; validated 2e-3"):
                        nc.vector.tensor_reduce(
                            u[:, b0:b0 + NI, :], iv,
                            axis=mybir.AxisListType.X, op=mybir.AluOpType.add)

                # ---- SSM: Bu = B^T.T @ u  (2 s-tiles x 4 chunks of 512)
                uf = u[:].rearrange("p a b -> p (a b)")
                bu = []
                for m in range(2):
                    pm = cps.tile([128, 2048], F32, tag="cps")
                    for j in range(4):
                        nc.tensor.matmul(
                            pm[:, 512 * j:512 * (j + 1)],
                            btw[:, 128 * m:128 * (m + 1)],
                            uf[:, 512 * j:512 * (j + 1)],
                            start=True, stop=True)
                    bu.append(pm)
                # sx_m[s,b] = sum_tau W[s,tau] * Bu[s,b,tau]
                sx = []
                for m in range(2):
                    tmp = big.tile([128, BL, L], F32, tag="tmp")
                    nc.vector.tensor_tensor(
                        tmp[:], bu[m][:].rearrange("p (a b) -> p a b", b=L),
                        wtw[:, m:m + 1, :].broadcast_to((128, BL, L)),
                        op=mybir.AluOpType.mult)
                    sxm = big.tile([128, BL], F32, tag=f"sx{m}")
                    nc.vector.tensor_reduce(
                        sxm[:], tmp[:], axis=mybir.AxisListType.X,
                        op=mybir.AluOpType.add)
                    sx.append(sxm)
                ub = big.tile([128, BL], F32)
                with nc.allow_low_precision(reason="sum of bf16 u in f32 out"):
                    nc.vector.tensor_reduce(
                        ub[:], u[:], axis=mybir.AxisListType.X,
                        op=mybir.AluOpType.add)

            with tc.tile_pool(name="tail", bufs=1, space="PSUM") as tps:
                # pooled[o,b] = Ct.T@sx0 + ... + Dt.T@ub   (+ h0 bias via ACT)
                pooled_s = []
                o2s = big.tile([64, S], F32)
                for m in range(2):
                    pp = tps.tile([128, BL], F32, tag=f"pl{m}")
                    ops = [(ctw[:, 0, 128 * m:128 * (m + 1)], sx[0]),
                           (ctw[:, 1, 128 * m:128 * (m + 1)], sx[1]),
                           (dtw[:, 128 * m:128 * (m + 1)], ub)]
                    for i, (lt_, rt) in enumerate(ops):
                        nc.tensor.matmul(pp[:], lt_, rt[:],
                                         start=(i == 0), stop=(i == 2))
                    ps_t = big.tile([128, BL], F32, tag=f"pooled{m}")
                    nc.scalar.activation(ps_t[:], pp[:], AF.Identity,
                                         bias=pbw[:, m:m + 1], scale=1.0)
                    pooled_s.append(ps_t)
                    # transpose to (b, o) for the activations output
                    ptr = tps.tile([64, 128], F32, tag="ptr", bufs=2)
                    nc.tensor.transpose(ptr[:], ps_t[:], ident[:])
                    nc.vector.tensor_copy(o2s[:, 128 * m:128 * (m + 1)], ptr[:])
                nc.sync.dma_start(out2_d[:, :], o2s[:])

                # head
                hp = tps.tile([128, BL], F32, tag="hp")
                for m in range(2):
                    nc.tensor.matmul(hp[:], w1w[:, m, :], pooled_s[m][:],
                                     start=(m == 0), stop=(m == 1))
                hs = big.tile([128, BL], F32)
                nc.scalar.activation(hs[:], hp[:], AF.Relu, bias=b1w[:], scale=1.0)
                lp = tps.tile([16, BL], F32, tag="lp")
                nc.tensor.matmul(lp[0:10, :], w2w[:], hs[:], start=True, stop=True)
                ls = big.tile([16, BL], F32)
                nc.scalar.activation(ls[0:10, :], lp[0:10, :], AF.Identity,
                                     bias=b2w[0:10, :], scale=1.0)
                lt = tps.tile([64, 16], F32, tag="lt")
                nc.tensor.transpose(lt[:, 0:10], ls[0:10, :], ident[0:10, 0:10])
                o1s = big.tile([64, 16], F32)
                nc.vector.tensor_copy(o1s[:, 0:10], lt[:, 0:10])
                nc.sync.dma_start(out1_d[:, :], o1s[:, 0:10])

    nc.finalize()
    return nc


def prep_in_maps(inputs):
    f32 = np.float32
    bf = ml_dtypes.bfloat16

    c1 = np.asarray(inputs["conv1_w"], dtype=f32)   # (64,3,3,3)
    c1T = np.zeros((128, 64), f32)
    for t, (dy, dx) in enumerate(TAPS):
        c1T[3 * t:3 * t + 3, :] = c1[:, :, dy + 1, dx + 1].T
    c2 = np.asarray(inputs["conv2_w"], dtype=f32)   # (128,64,3,3)
    # 5 K=128 weight blocks; block q pairs (lower tap, upper tap):
    #  q=0..2: {(dy,-1) lower, (dy,0) upper(+1)} on fm1, window 34*dy-1
    #  q=3:    {(-1,+1) lower, (0,+1) upper(+34)} on scr, window -33
    #  q=4:    {(1,+1) lower, zero upper} on scr, window +35
    c2q = np.zeros((5, 128, 128), f32)
    for i, dy in enumerate((-1, 0, 1)):
        c2q[i, 0:64, :] = c2[:, :, dy + 1, 0].T
        c2q[i, 64:128, :] = c2[:, :, dy + 1, 1].T
    c2q[3, 0:64, :] = c2[:, :, 0, 2].T
    c2q[3, 64:128, :] = c2[:, :, 1, 2].T
    c2q[4, 0:64, :] = c2[:, :, 2, 2].T
    c3 = np.asarray(inputs["conv3_w"], dtype=f32)
    c3T = np.zeros((9, 128, 128), f32)
    for t, (dy, dx) in enumerate(TAPS):
        c3T[t] = c3[:, :, dy + 1, dx + 1].T

    scb = {}
    for i in (1, 2, 3):
        g = np.asarray(inputs[f"bn{i}_g"], f32)
        b = np.asarray(inputs[f"bn{i}_b"], f32)
        m = np.asarray(inputs[f"bn{i}_m"], f32)
        v = np.asarray(inputs[f"bn{i}_v"], f32)
        inv = g / np.sqrt(v + np.float32(1e-5))
        scb[i] = (inv.astype(f32), (b - m * inv).astype(f32))

    A = -np.log1p(np.exp(np.asarray(inputs["ssm_A"], np.float64)))
    wts = np.stack([(1.0 - A ** (L - t)) / (1.0 - A) for t in range(L)], 1)  # (S,L)
    Wt = (wts / (32.0 * L)).astype(f32).reshape(2, 128, L).transpose(1, 0, 2)
    Wt = np.ascontiguousarray(Wt)  # (128, 2, L): [p, m, tau] = s=128m+p
    BT = np.ascontiguousarray(np.asarray(inputs["ssm_B"], f32).T)  # (128,256)
    Cm = np.asarray(inputs["ssm_C"], f32)
    Ct = np.ascontiguousarray(Cm.T.reshape(2, 128, S))
    Dt = np.ascontiguousarray((np.asarray(inputs["ssm_D"], np.float64).T / (32.0 * L)).astype(f32))
    h0 = np.asarray(inputs["ssm_h0"], np.float64)
    geo = A * (1.0 - A ** L) / (1.0 - A)
    pbias = ((Cm.astype(np.float64) @ (geo * h0)) / L).astype(f32).reshape(2, 128).T
    pbias = np.ascontiguousarray(pbias)  # (128, 2)

    w1T = np.ascontiguousarray(np.asarray(inputs["head_w1"], f32).T.reshape(2, 128, 128))
    w2T = np.ascontiguousarray(np.asarray(inputs["head_w2"], f32).T)
    hb1 = np.asarray(inputs["head_b1"], f32)
    hb2 = np.asarray(inputs["head_b2"], f32)

    shared = dict(c1T=c1T.astype(bf), c2q=c2q.astype(bf), c3T=c3T.astype(bf),
                  inv1=scb[1][0], beta1=scb[1][1],
                  inv2=scb[2][0], beta2=scb[2][1],
                  inv3=scb[3][0], beta3=scb[3][1],
                  BT=BT.astype(bf), Wt=Wt, Ct=Ct, Dt=Dt, w1T=w1T, w2T=w2T,
                  hb1=hb1, hb2=hb2, pbias=pbias)

    # host-staged x: (SUB, 32, W) per core, 9 shifted tap replicas of the
    # zero-padded 34x34 frame layout
    x = np.asarray(inputs["x"], f32)
    in_maps = []
    for i in range(NCORES):
        xc = x[i * BL:(i + 1) * BL].reshape(SUB, NI, 3, 32, 32)
        wide = np.zeros((SUB, 3, W + 70), f32)
        wv = wide[:, :, 35 + G:35 + G + NI * FR].reshape(SUB, 3, NI, 34, 34)
        wv[:, :, :, 1:33, 1:33] = xc.transpose(0, 2, 1, 3, 4)
        xst = np.zeros((SUB, 32, W), f32)
        for t, (dy, dx) in enumerate(TAPS):
            d = 34 * dy + dx
            xst[:, 3 * t:3 * t + 3, :] = wide[:, :, 35 + d:35 + d + W]
        m = dict(shared)
        m["xst"] = np.ascontiguousarray(xst.astype(bf))
        in_maps.append(m)
    return in_maps


_NC_CACHE = []


def kernel(**inputs):
    if not _NC_CACHE:
        _NC_CACHE.append(build())
    nc = _NC_CACHE[0]
    in_maps = prep_in_maps(inputs)
    res = run_bass_kernel_spmd(nc, in_maps, core_ids=list(range(NCORES)))
    out = np.concatenate([res.results[i]["out1"] for i in range(NCORES)], axis=0)
    act = np.concatenate([res.results[i]["out2"] for i in range(NCORES)], axis=0)
    return out.astype(np.float32), act.astype(np.float32)
